# revision 20
# baseline (speedup 1.0000x reference)
"""Distributed MHA forward for trn2 (8 NeuronCores).

Problem: x[4,2048,1024] -> MHA(16 heads, dh=64) -> [4,2048,1024], fp32 I/O.

Sharding: core (b, g) = batch b (4) x head-group g (2 groups of 8 heads).
Each core computes q,k,v projections for its 8 heads, attention, and the
partial out-projection ctx_g @ Wo[g*512:(g+1)*512, :].  A pair-wise
ReduceScatter {2b, 2b+1} sums the partials and leaves rows [g*1024:(g+1)*1024]
on core 2b+g.  Host concatenates and adds the (bv @ Wo + bo) bias (bv folds
out of attention because softmax rows sum to 1).

Device layout (per core, all bf16 compute with fp32 PSUM accumulation):
  xT   [1024, 2048] = x[b].T          (host-transposed)
  qT,kT [512, 2048] via lhsT=Wq-slice, rhs=xT   (features on partitions)
  v    [2048, 520]  natural, 65-wide per head with a ones column at e=64
                    -> PV matmul emits the softmax denominator for free
  scoresT [nk, nq] per head via lhsT=kT-chunk (K=64), rhs=qT; head pairs at
                    base partitions 0/64 run concurrently on the PE
  exp on ScalarE from 2-bank PSUM groups, bf16 out
  ctxT = (v_aug^T @ exps) / den     [512, 2048] feature-major
  out  = ctxT^T @ Wo-slice          [2048, 1024] fp32 partial -> ReduceScatter
"""

import os
import numpy as np
import ml_dtypes

import concourse.bass as bass
import concourse.mybir as mybir
import concourse.tile as tile
from concourse import bacc
from concourse.bass_utils import run_bass_kernel_spmd

B, N, D = 4, 2048, 1024
H, DH, P = 16, 64, 128
HG = 8            # heads per core
GF = HG * DH      # 512 features per head-group
KO = D // P       # 8 k-blocks over model dim
FO = GF // P      # 4 feature blocks of the group
NKC = N // P      # 16 nk chunks
NQB = 512         # nq block
NQBS = N // NQB   # 4
NCORES = 8

F32 = mybir.dt.float32
BF16 = mybir.dt.bfloat16
BF16_NP = ml_dtypes.bfloat16

NORM_METHOD = os.environ.get("BASS_NORM_METHOD", "gpsimd")  # gpsimd | dma

LAST_RESULTS = None  # BassKernelResults of the most recent run (for test.py)


def _build_nc():
    nc = bacc.Bacc(
        "TRN2",
        target_bir_lowering=False,
        debug=False,
        num_devices=NCORES,
    )
    xT = nc.dram_tensor("xT", [D, N], BF16, kind="ExternalInput")
    wq = nc.dram_tensor("wq", [D, GF], BF16, kind="ExternalInput")
    wk = nc.dram_tensor("wk", [D, GF], BF16, kind="ExternalInput")
    wv = nc.dram_tensor("wv", [D, GF], BF16, kind="ExternalInput")
    wo = nc.dram_tensor("wo", [GF, D], BF16, kind="ExternalInput")
    bq8 = nc.dram_tensor("bq8", [GF], BF16, kind="ExternalInput")
    bk = nc.dram_tensor("bk", [GF], BF16, kind="ExternalInput")
    out_ext = nc.dram_tensor("out", [N // 2, D], BF16, kind="ExternalOutput")

    with tile.TileContext(nc) as tc:
        _build_body(nc, tc, xT, wq, wk, wv, wo, bq8, bk, out_ext)
    nc.finalize()
    return nc


def _build_body(nc, tc, xT, wq, wk, wv, wo, bq8, bk, out_ext):
    mm = nc.tensor.matmul
    Exp = mybir.ActivationFunctionType.Exp

    with (
        tc.tile_pool(name="persist", bufs=1) as pers,
        tc.tile_pool(name="dram", bufs=1, space="DRAM") as dram,
    ):
        # per-fblock q/k tiles so attention on head pair 0 can start while
        # later fblocks are still projecting
        qT_f = [pers.tile([P, N], BF16, name=f"qT_f{fc}") for fc in range(FO)]
        kT_f = [pers.tile([P, N], BF16, name=f"kT_f{fc}") for fc in range(FO)]
        v_sb = pers.tile([P, NKC, HG * (DH + 1)], BF16, name="v_sb")
        ctxT_sb = pers.tile([P, FO, N], BF16, name="ctxT_sb")
        wo_sb = pers.tile([P, FO, D], BF16, name="wo_sb")
        bq8_sb = pers.tile([1, GF], BF16, name="bq8_sb")
        bk_sb = pers.tile([1, GF], BF16, name="bk_sb")
        ones_row = pers.tile([1, N], BF16, name="ones_row")

        # One RS chunk per row block: the pair splits each block's 512 rows
        # (rank0 keeps the first 256, rank1 the second 256); the host remaps.
        # Each block's ReduceScatter issues right after its out-projection and
        # overlaps the next block's compute.
        chunks = [
            dram.tile([NQB, D], BF16, name=f"chunk{q}") for q in range(NQBS)
        ]
        rs_outs = [
            dram.tile([NQB // 2, D], BF16, name=f"rs{q}") for q in range(NQBS)
        ]
        recip_drams = [
            [dram.tile([2, NQB], F32, name=f"recip_d{q}_{hp}") for hp in range(4)]
            for q in range(NQBS)
        ]

        nc.sync.dma_start(wo_sb[:], wo.rearrange("(fo p) d -> p fo d", p=P))
        nc.sync.dma_start(bq8_sb[:], bq8[None, :])
        nc.sync.dma_start(bk_sb[:], bk[None, :])
        nc.vector.memset(ones_row[:], 1.0)

        # Attention-scope pools are opened first; the phase-1 pools live in a
        # nested ExitStack that closes after the last projection so the
        # out-projection PSUM pool can reuse those banks.
        from contextlib import ExitStack

        att_es = ExitStack()
        exps_pool = att_es.enter_context(tc.tile_pool(name="exps", bufs=2))
        psum_s = att_es.enter_context(tc.tile_pool(name="ps_sc", bufs=2, space="PSUM"))
        psum_pv = att_es.enter_context(tc.tile_pool(name="ps_pv", bufs=2, space="PSUM"))
        misc = att_es.enter_context(tc.tile_pool(name="att_misc", bufs=4))
        den_pool = att_es.enter_context(tc.tile_pool(name="den_pool", bufs=2))
        outsb = att_es.enter_context(tc.tile_pool(name="out_sb", bufs=4))

        def attention_block(nqb, hp):
            """scoresT + exp + PV + unnormalized ctx evac + per-pair
            normalization for one (row block, head pair)."""
            qsl = slice(nqb * NQB, (nqb + 1) * NQB)
            den_pair = den_pool.tile([2, NQB], F32, tag="den", name="den_pair")
            exps = [
                exps_pool.tile(
                    [P, NKC, NQB], BF16, tag="exps", name=f"exps{i}", bufs=3
                )
                for i in range(2)
            ]
            # 2 nk-chunks per PSUM group; head pair interleaved so the K=64
            # matmuls pack PE row groups 0/64
            for grp in range(NKC // 2):
                pss = [
                    psum_s.tile([P, 2 * NQB], F32, tag="sc", name="ps_sc")
                    for _ in range(2)
                ]
                for j in range(2):
                    nkc = grp * 2 + j
                    ksl = slice(nkc * P, (nkc + 1) * P)
                    for i in range(2):
                        rows = slice(i * 64, (i + 1) * 64)
                        mm(
                            pss[i][:, j * NQB : (j + 1) * NQB],
                            kT_f[hp][rows, ksl],
                            qT_f[hp][rows, qsl],
                            start=True,
                            stop=True,
                        )
                for i in range(2):
                    nc.scalar.activation(
                        exps[i][:, grp * 2 : grp * 2 + 2, :],
                        pss[i].rearrange("p (c n) -> p c n", n=NQB),
                        Exp,
                    )
            # PV: ctxT_aug[65, nq] per head; den lands on PSUM partition 64
            for i in range(2):
                hl = 2 * hp + i
                ps_pv = psum_pv.tile([DH + 1, NQB], F32, tag="pv", name="ps_pv")
                for nkc in range(NKC):
                    mm(
                        ps_pv[:],
                        v_sb[:, nkc, hl * 65 : (hl + 1) * 65],
                        exps[i][:, nkc, :],
                        start=(nkc == 0),
                        stop=(nkc == NKC - 1),
                    )
                if i == 0:
                    nc.vector.tensor_copy(ctxT_sb[0:64, hp, qsl], ps_pv[0:DH, :])
                else:
                    ctmp = misc.tile([64, NQB], BF16, tag="ctmp", name="ctmp", bufs=2)
                    nc.vector.tensor_copy(ctmp[:], ps_pv[0:DH, :])
                    nc.sync.dma_start(ctxT_sb[64:128, hp, qsl], ctmp[:])
                # den row: PSUM partition 64 -> SBUF partition 64 (same lane),
                # then DMA shifts it into den_pair[i]
                dstage = misc.tile([65, NQB], F32, tag="dstage", name="dstage", bufs=2)
                nc.vector.tensor_copy(dstage[64:65, :], ps_pv[64:65, :])
                nc.sync.dma_start(den_pair[i : i + 1, :], dstage[64:65, :])
            # normalize this pair in place; overlaps the next pair's compute
            qsl2 = qsl
            recip_pair = den_pool.tile([2, NQB], F32, tag="recip", name="recip_pair")
            nc.vector.reciprocal(recip_pair[:], den_pair[:])
            rd = recip_drams[nqb][hp]
            nc.sync.dma_start(rd[:, :], recip_pair[:])
            rbc = misc.tile([P, NQB], F32, tag="rbc", name="rbc", bufs=2)
            nc.sync.dma_start(rbc[0:64, :], rd[0:1, :].to_broadcast((64, NQB)))
            nc.sync.dma_start(rbc[64:128, :], rd[1:2, :].to_broadcast((64, NQB)))
            nc.vector.tensor_tensor(
                ctxT_sb[:, hp, qsl2],
                ctxT_sb[:, hp, qsl2],
                rbc[:],
                mybir.AluOpType.mult,
            )

        def finish_block(nqb, psum_o):
            """out projection of one row block into its RS chunk."""
            chunk = chunks[nqb]
            row0 = 0
            for lq in range(NQB // P):
                nqc = nqb * (NQB // P) + lq
                for cb in range(D // NQB):
                    ps = psum_o.tile([P, NQB], F32, tag="o", name="ps_o")
                    for fc in range(FO):
                        mm(
                            ps[:],
                            ctxT_sb[:, fc, nqc * P : (nqc + 1) * P],
                            wo_sb[:, fc, cb * NQB : (cb + 1) * NQB],
                            start=(fc == 0),
                            stop=(fc == FO - 1),
                        )
                    ob = outsb.tile([P, NQB], BF16, tag="ob", name="ob", bufs=2)
                    nc.vector.tensor_copy(ob[:], ps[:])
                    nc.sync.dma_start(
                        chunk[
                            row0 + lq * P : row0 + (lq + 1) * P,
                            cb * NQB : (cb + 1) * NQB,
                        ],
                        ob[:],
                    )

        def reduce_scatter(q):
            nc.gpsimd.collective_compute(
                "ReduceScatter",
                mybir.AluOpType.add,
                replica_groups=[[0, 1], [2, 3], [4, 5], [6, 7]],
                ins=[chunks[q].opt()],
                outs=[rs_outs[q].opt()],
            )
            # gpsimd queue: a sync-queue DMA here would head-of-line block all
            # later sync DMAs behind the RS wait
            nc.gpsimd.dma_start(
                out_ext[q * (NQB // 2) : (q + 1) * (NQB // 2), :], rs_outs[q][:]
            )

        # ---- Phase 1 (projections), interleaved with block-0 attention ----
        ph1_es = ExitStack()
        ph1 = ph1_es.enter_context(tc.tile_pool(name="ph1", bufs=1))
        psum1 = ph1_es.enter_context(
            tc.tile_pool(name="ph1_psum", bufs=2, space="PSUM")
        )
        # per-ko xT tiles: the first projection matmul only waits for the
        # first 512 KB slice instead of the whole 4 MB load
        wk_sb = ph1.tile([P, KO, GF], BF16, name="wk_sb")
        nc.sync.dma_start(wk_sb[:], wk.rearrange("(ko p) f -> p ko f", p=P))
        xT4 = xT.rearrange("(ko p) n -> ko p n", p=P)
        xT_k = []
        for ko in range(KO):
            t = ph1.tile([P, N], BF16, name=f"xT_k{ko}")
            nc.sync.dma_start(t[:], xT4[ko])
            xT_k.append(t)
        wq_sb = ph1.tile([P, KO, GF], BF16, name="wq_sb")
        nc.sync.dma_start(wq_sb[:], wq.rearrange("(ko p) f -> p ko f", p=P))
        wv_sb = ph1.tile([P, KO, GF], BF16, name="wv_sb")
        nc.sync.dma_start(wv_sb[:], wv.rearrange("(ko p) f -> p ko f", p=P))

        def proj_qk(fc):
            # bias rides a K=1 matmul chunk (lhsT = bias row, rhs = ones row);
            # the q-side 1/8 scale is folded into wq/bq8 on the host
            for w_sb, b_sb, dst in (
                (wk_sb, bk_sb, kT_f[fc]),
                (wq_sb, bq8_sb, qT_f[fc]),
            ):
                for nb in range(NQBS):
                    ps = psum1.tile([P, NQB], F32, tag="proj", name="ps_proj")
                    for ko in range(KO):
                        mm(
                            ps[:],
                            w_sb[:, ko, fc * P : (fc + 1) * P],
                            xT_k[ko][:, nb * NQB : (nb + 1) * NQB],
                            start=(ko == 0),
                            stop=False,
                        )
                    mm(
                        ps[:],
                        b_sb[:, fc * P : (fc + 1) * P],
                        ones_row[:, nb * NQB : (nb + 1) * NQB],
                        start=False,
                        stop=True,
                    )
                    nc.vector.tensor_copy(
                        dst[:, nb * NQB : (nb + 1) * NQB], ps[:]
                    )

        proj_qk(0)
        # v natural: [n, 512] = xT-chunk^T @ Wv, 65-wide per head (ones col)
        v4 = v_sb.rearrange("p nk (h e) -> p nk h e", e=DH + 1)
        nc.vector.memset(v4[:, :, :, DH], 1.0)
        for nk in range(NKC):
            ps = psum1.tile([P, GF], F32, tag="proj", name="ps_v")
            for ko in range(KO):
                mm(
                    ps[:],
                    xT_k[ko][:, nk * P : (nk + 1) * P],
                    wv_sb[:, ko, :],
                    start=(ko == 0),
                    stop=(ko == KO - 1),
                )
            nc.vector.tensor_copy(
                v4[:, nk, :, :DH],
                ps.rearrange("p (h e) -> p h e", e=DH),
            )

        for hp in range(HG // 2):
            attention_block(0, hp)
            if hp + 1 < FO:
                proj_qk(hp + 1)
        ph1_es.close()

        # out-projection PSUM pool opens after the projection pool closed
        psum_o = att_es.enter_context(
            tc.tile_pool(name="out_ps", bufs=2, space="PSUM")
        )
        finish_block(0, psum_o)
        reduce_scatter(0)

        for nqb in range(1, NQBS):
            for hp in range(HG // 2):
                attention_block(nqb, hp)
            finish_block(nqb, psum_o)
            reduce_scatter(nqb)
        att_es.close()


_NC_CACHE = None


def _get_nc():
    global _NC_CACHE
    if _NC_CACHE is None:
        _NC_CACHE = _build_nc()
    return _NC_CACHE


# --------------------------------------------------------------------------
# Timing support (test-only): build the sharded jit once, rerun on
# device-resident inputs, and subtract the axon dispatch floor measured on a
# trivial kernel.
# --------------------------------------------------------------------------


def _build_trivial_nc():
    nc = bacc.Bacc("TRN2", target_bir_lowering=False, debug=False,
                   num_devices=NCORES)
    tin = nc.dram_tensor("tin", [P, P], F32, kind="ExternalInput")
    tout = nc.dram_tensor("tout", [P, P], F32, kind="ExternalOutput")
    with tile.TileContext(nc) as tc:
        with tc.tile_pool(name="t", bufs=1) as pool:
            t = pool.tile([P, P], F32, name="t")
            nc.sync.dma_start(t[:], tin[:, :])
            nc.sync.dma_start(tout[:, :], t[:])
    nc.finalize()
    return nc


class _SpmdRunner:
    """Mirror of bass2jax.run_bass_via_pjrt's multi-core path with a cached
    jit so repeat executions don't recompile."""

    def __init__(self, nc):
        import jax
        from jax.sharding import Mesh, PartitionSpec
        try:
            from jax.experimental.shard_map import shard_map
        except ImportError:
            from jax.shard_map import shard_map
        from concourse import bass2jax as b2j

        b2j.install_neuronx_cc_hook()
        self.nc = nc
        partition_name = (
            nc.partition_id_tensor.name if nc.partition_id_tensor else None
        )
        in_names, out_names, out_avals, zero_outs = [], [], [], []
        for alloc in nc.m.functions[0].allocations:
            if not isinstance(alloc, mybir.MemoryLocationSet):
                continue
            name = alloc.memorylocations[0].name
            if alloc.kind == "ExternalInput":
                if name != partition_name:
                    in_names.append(name)
            elif alloc.kind == "ExternalOutput":
                shape = tuple(alloc.tensor_shape)
                dtype = mybir.dt.np(alloc.dtype)
                out_names.append(name)
                out_avals.append(jax.core.ShapedArray(shape, dtype))
                zero_outs.append(np.zeros(shape, dtype))
        self.n_params = len(in_names)
        n_outs = len(out_avals)
        in_names = in_names + out_names
        if partition_name is not None:
            in_names.append(partition_name)
        self.in_names = in_names
        self.out_names = out_names
        self.out_avals = out_avals
        self.zero_outs = zero_outs

        def _body(*args):
            operands = list(args)
            if partition_name is not None:
                operands.append(b2j.partition_id_tensor())
            outs = b2j._bass_exec_p.bind(
                *operands,
                out_avals=tuple(out_avals),
                in_names=tuple(in_names),
                out_names=tuple(out_names),
                lowering_input_output_aliases=(),
                sim_require_finite=True,
                sim_require_nnan=True,
                nc=nc,
            )
            return tuple(outs)

        devices = jax.devices()[:NCORES]
        self.mesh = Mesh(np.asarray(devices), ("core",))
        in_specs = (PartitionSpec("core"),) * (self.n_params + n_outs)
        out_specs = (PartitionSpec("core"),) * n_outs
        self.fn = jax.jit(
            shard_map(_body, mesh=self.mesh, in_specs=in_specs,
                      out_specs=out_specs, check_rep=False),
            keep_unused=True,
        )
        self._jax = jax

    def make_fn_k(self, K):
        """jit that executes the NEFF K times; all outputs kept live so the
        calls can't be DCE'd. Used to amortize the ~78 ms axon dispatch floor
        out of timing: exec_ns ~= (t(K) - t(1)) / (K - 1)."""
        import jax
        from jax.sharding import PartitionSpec
        try:
            from jax.experimental.shard_map import shard_map
        except ImportError:
            from jax.shard_map import shard_map
        from concourse import bass2jax as b2j

        nc = self.nc
        partition_name = nc.partition_id_tensor.name if nc.partition_id_tensor else None
        in_names, out_names, out_avals = self.in_names, self.out_names, self.out_avals

        def _body_k(*args):
            all_outs = []
            for _ in range(K):
                operands = list(args)
                if partition_name is not None:
                    operands.append(b2j.partition_id_tensor())
                outs = b2j._bass_exec_p.bind(
                    *operands,
                    out_avals=tuple(out_avals),
                    in_names=tuple(in_names),
                    out_names=tuple(out_names),
                    lowering_input_output_aliases=(),
                    sim_require_finite=True,
                    sim_require_nnan=True,
                    nc=nc,
                )
                all_outs.extend(outs)
            return tuple(all_outs)

        n_outs = len(out_avals)
        in_specs = (PartitionSpec("core"),) * (self.n_params + n_outs)
        out_specs = (PartitionSpec("core"),) * (n_outs * K)
        return jax.jit(
            shard_map(_body_k, mesh=self.mesh, in_specs=in_specs,
                      out_specs=out_specs, check_rep=False),
            keep_unused=True,
        )

    def time_k(self, in_maps, K=8, reps=12):
        import time as _time

        dev_in, dev_zero = self.prepare(in_maps)
        fn_k = self.make_fn_k(K)
        fn_1 = self.make_fn_k(1)
        for fn in (fn_1, fn_k):
            self._jax.block_until_ready(fn(*dev_in, *dev_zero))  # compile+warm
        t1s, tks = [], []
        for _ in range(reps):
            t0 = _time.perf_counter()
            self._jax.block_until_ready(fn_1(*dev_in, *dev_zero))
            t1s.append(_time.perf_counter() - t0)
            t0 = _time.perf_counter()
            self._jax.block_until_ready(fn_k(*dev_in, *dev_zero))
            tks.append(_time.perf_counter() - t0)
        t1, tk = min(t1s), min(tks)
        return (tk - t1) / (K - 1), t1, tk

    def _shard(self, arrs):
        import jax
        from jax.sharding import NamedSharding, PartitionSpec

        sh = NamedSharding(self.mesh, PartitionSpec("core"))
        return [jax.device_put(a, sh) for a in arrs]

    def prepare(self, in_maps):
        concat_in = [
            np.concatenate([np.asarray(m[name]) for m in in_maps], axis=0)
            for name in self.in_names[: self.n_params]
        ]
        concat_zeros = [
            np.zeros((NCORES * z.shape[0], *z.shape[1:]), z.dtype)
            for z in self.zero_outs
        ]
        return self._shard(concat_in), self._shard(concat_zeros)

    def run(self, dev_in, dev_zero):
        outs = self.fn(*dev_in, *dev_zero)
        self._jax.block_until_ready(outs)
        return outs

    def time(self, in_maps, reps=10):
        import time as _time

        dev_in, dev_zero = self.prepare(in_maps)
        self.run(dev_in, dev_zero)  # warm/compile
        ts = []
        for _ in range(reps):
            t0 = _time.perf_counter()
            self.run(dev_in, dev_zero)
            ts.append(_time.perf_counter() - t0)
        return min(ts), ts

    def results(self, in_maps):
        dev_in, dev_zero = self.prepare(in_maps)
        outs = self.run(dev_in, dev_zero)
        res = []
        for c in range(NCORES):
            res.append(
                {
                    name: np.asarray(outs[i]).reshape(
                        NCORES, *self.out_avals[i].shape
                    )[c]
                    for i, name in enumerate(self.out_names)
                }
            )
        return res


_RUNNER = None
_TRIVIAL_RUNNER = None


def get_runner():
    global _RUNNER
    if _RUNNER is None:
        _RUNNER = _SpmdRunner(_get_nc())
    return _RUNNER


def get_trivial_runner():
    global _TRIVIAL_RUNNER
    if _TRIVIAL_RUNNER is None:
        _TRIVIAL_RUNNER = _SpmdRunner(_build_trivial_nc())
    return _TRIVIAL_RUNNER


def make_in_maps(x, Wq, bq, Wk, bk, Wv, bv, Wo, bo):
    x = np.asarray(x, np.float32)
    in_maps = []
    for core in range(NCORES):
        b, g = core // 2, core % 2
        gsl = slice(g * GF, (g + 1) * GF)
        in_maps.append(
            {
                "xT": np.ascontiguousarray(x[b].T).astype(BF16_NP),
                "wq": np.ascontiguousarray(np.asarray(Wq)[:, gsl] / 8.0).astype(
                    BF16_NP
                ),
                "wk": np.ascontiguousarray(np.asarray(Wk)[:, gsl]).astype(BF16_NP),
                "wv": np.ascontiguousarray(np.asarray(Wv)[:, gsl]).astype(BF16_NP),
                "wo": np.ascontiguousarray(np.asarray(Wo)[gsl, :]).astype(BF16_NP),
                "bq8": np.ascontiguousarray(np.asarray(bq)[gsl] / 8.0).astype(
                    BF16_NP
                ),
                "bk": np.ascontiguousarray(np.asarray(bk)[gsl]).astype(BF16_NP),
            }
        )
    return in_maps


def kernel(x, Wq, bq, Wk, bk, Wv, bv, Wo, bo):
    Wo = np.asarray(Wo, np.float32)
    bv = np.asarray(bv, np.float32)
    bo = np.asarray(bo, np.float32)
    in_maps = make_in_maps(x, Wq, bq, Wk, bk, Wv, bv, Wo, bo)
    results = get_runner().results(in_maps)

    post = (bv @ Wo + bo).astype(np.float32)  # softmax rows sum to 1 -> bv folds
    # Per-block RS: core 2b+g's out rows [q*256:(q+1)*256] are the global rows
    # [q*512 + g*256 : q*512 + (g+1)*256] of batch b.
    half = NQB // 2
    out = np.empty((B, N, D), np.float32)
    for b in range(B):
        for g in range(2):
            r = np.asarray(results[2 * b + g]["out"], np.float32)
            for q in range(NQBS):
                out[b, q * NQB + g * half : q * NQB + (g + 1) * half] = r[
                    q * half : (q + 1) * half
                ]
        out[b] += post
    return out


# revision 21
# speedup vs baseline: 2.5950x; 2.5950x over previous
"""Distributed MHA forward for trn2 (8 NeuronCores).

Problem: x[4,2048,1024] -> MHA(16 heads, dh=64) -> [4,2048,1024], fp32 I/O.

Sharding: core (b, g) = batch b (4) x head-group g (2 groups of 8 heads).
Each core computes q,k,v projections for its 8 heads, attention, and the
partial out-projection ctx_g @ Wo[g*512:(g+1)*512, :].  A pair-wise
ReduceScatter {2b, 2b+1} sums the partials and leaves rows [g*1024:(g+1)*1024]
on core 2b+g.  Host concatenates and adds the (bv @ Wo + bo) bias (bv folds
out of attention because softmax rows sum to 1).

Device layout (per core, all bf16 compute with fp32 PSUM accumulation):
  xT   [1024, 2048] = x[b].T          (host-transposed)
  qT,kT [512, 2048] via lhsT=Wq-slice, rhs=xT   (features on partitions)
  v    [2048, 520]  natural, 65-wide per head with a ones column at e=64
                    -> PV matmul emits the softmax denominator for free
  scoresT [nk, nq] per head via lhsT=kT-chunk (K=64), rhs=qT; head pairs at
                    base partitions 0/64 run concurrently on the PE
  exp on ScalarE from 2-bank PSUM groups, bf16 out
  ctxT = (v_aug^T @ exps) / den     [512, 2048] feature-major
  out  = ctxT^T @ Wo-slice          [2048, 1024] fp32 partial -> ReduceScatter
"""

import numpy as np
import ml_dtypes

import concourse.mybir as mybir
import concourse.tile as tile
from concourse import bacc

B, N, D = 4, 2048, 1024
H, DH, P = 16, 64, 128
HG = 8            # heads per core
GF = HG * DH      # 512 features per head-group
KO = D // P       # 8 k-blocks over model dim
FO = GF // P      # 4 feature blocks of the group
NKC = N // P      # 16 nk chunks
NQB = 512         # nq block
NQBS = N // NQB   # 4
NCORES = 8

F32 = mybir.dt.float32
BF16 = mybir.dt.bfloat16
BF16_NP = ml_dtypes.bfloat16


def _build_nc():
    nc = bacc.Bacc(
        "TRN2",
        target_bir_lowering=False,
        debug=False,
        num_devices=NCORES,
    )
    xT = nc.dram_tensor("xT", [D, N], BF16, kind="ExternalInput")
    wq = nc.dram_tensor("wq", [D, GF], BF16, kind="ExternalInput")
    wk = nc.dram_tensor("wk", [D, GF], BF16, kind="ExternalInput")
    wv = nc.dram_tensor("wv", [D, GF], BF16, kind="ExternalInput")
    wo = nc.dram_tensor("wo", [GF, D], BF16, kind="ExternalInput")
    bq8 = nc.dram_tensor("bq8", [GF], BF16, kind="ExternalInput")
    bk = nc.dram_tensor("bk", [GF], BF16, kind="ExternalInput")
    out_ext = nc.dram_tensor("out", [N // 2, D], BF16, kind="ExternalOutput")

    with tile.TileContext(nc) as tc:
        _build_body(nc, tc, xT, wq, wk, wv, wo, bq8, bk, out_ext)
    nc.finalize()
    return nc


def _build_body(nc, tc, xT, wq, wk, wv, wo, bq8, bk, out_ext):
    mm = nc.tensor.matmul
    Exp = mybir.ActivationFunctionType.Exp

    with (
        tc.tile_pool(name="persist", bufs=1) as pers,
        tc.tile_pool(name="dram", bufs=1, space="DRAM") as dram,
    ):
        # per-fblock q/k tiles so attention on head pair 0 can start while
        # later fblocks are still projecting
        qT_f = [pers.tile([P, N], BF16, name=f"qT_f{fc}") for fc in range(FO)]
        kT_f = [pers.tile([P, N], BF16, name=f"kT_f{fc}") for fc in range(FO)]
        v_sb = pers.tile([P, NKC, HG * (DH + 1)], BF16, name="v_sb")
        ctxT_sb = pers.tile([P, FO, N], BF16, name="ctxT_sb")
        wo_sb = pers.tile([P, FO, D], BF16, name="wo_sb")
        bq8_sb = pers.tile([1, GF], BF16, name="bq8_sb")
        bk_sb = pers.tile([1, GF], BF16, name="bk_sb")
        ones_row = pers.tile([1, N], BF16, name="ones_row")

        # One RS chunk per row block: the pair splits each block's 512 rows
        # (rank0 keeps the first 256, rank1 the second 256); the host remaps.
        # Each block's ReduceScatter issues right after its out-projection and
        # overlaps the next block's compute.
        # last block's RS is split in two so only a ~quarter-size collective
        # remains exposed after the final out-projection rows
        chunk_rows = [NQB, NQB, NQB, NQB // 2, NQB // 2]
        chunks = [
            dram.tile([r, D], BF16, name=f"chunk{q}")
            for q, r in enumerate(chunk_rows)
        ]
        rs_outs = [
            dram.tile([r // 2, D], BF16, name=f"rs{q}")
            for q, r in enumerate(chunk_rows)
        ]
        recip_drams = [
            [dram.tile([2, NQB], F32, name=f"recip_d{q}_{hp}") for hp in range(4)]
            for q in range(NQBS)
        ]

        nc.sync.dma_start(wo_sb[:], wo.rearrange("(fo p) d -> p fo d", p=P))
        nc.sync.dma_start(bq8_sb[:], bq8[None, :])
        nc.sync.dma_start(bk_sb[:], bk[None, :])
        nc.vector.memset(ones_row[:], 1.0)

        # Attention-scope pools are opened first; the phase-1 pools live in a
        # nested ExitStack that closes after the last projection so the
        # out-projection PSUM pool can reuse those banks.
        from contextlib import ExitStack

        att_es = ExitStack()
        exps_pool = att_es.enter_context(tc.tile_pool(name="exps", bufs=2))
        psum_s = att_es.enter_context(tc.tile_pool(name="ps_sc", bufs=2, space="PSUM"))
        psum_pv = att_es.enter_context(tc.tile_pool(name="ps_pv", bufs=2, space="PSUM"))
        misc = att_es.enter_context(tc.tile_pool(name="att_misc", bufs=4))
        den_pool = att_es.enter_context(tc.tile_pool(name="den_pool", bufs=2))
        outsb = att_es.enter_context(tc.tile_pool(name="out_sb", bufs=4))

        def attention_block(nqb, hp):
            """scoresT + exp + PV + unnormalized ctx evac + per-pair
            normalization for one (row block, head pair)."""
            qsl = slice(nqb * NQB, (nqb + 1) * NQB)
            den_pair = den_pool.tile([2, NQB], F32, tag="den", name="den_pair")
            exps = [
                exps_pool.tile(
                    [P, NKC, NQB], BF16, tag="exps", name=f"exps{i}", bufs=3
                )
                for i in range(2)
            ]
            # 2 nk-chunks per PSUM group; head pair interleaved so the K=64
            # matmuls pack PE row groups 0/64
            for grp in range(NKC // 2):
                pss = [
                    psum_s.tile([P, 2 * NQB], F32, tag="sc", name="ps_sc")
                    for _ in range(2)
                ]
                for j in range(2):
                    nkc = grp * 2 + j
                    ksl = slice(nkc * P, (nkc + 1) * P)
                    for i in range(2):
                        rows = slice(i * 64, (i + 1) * 64)
                        mm(
                            pss[i][:, j * NQB : (j + 1) * NQB],
                            kT_f[hp][rows, ksl],
                            qT_f[hp][rows, qsl],
                            start=True,
                            stop=True,
                        )
                for i in range(2):
                    nc.scalar.activation(
                        exps[i][:, grp * 2 : grp * 2 + 2, :],
                        pss[i].rearrange("p (c n) -> p c n", n=NQB),
                        Exp,
                    )
            # PV: ctxT_aug[65, nq] per head; den lands on PSUM partition 64
            for i in range(2):
                hl = 2 * hp + i
                ps_pv = psum_pv.tile([DH + 1, NQB], F32, tag="pv", name="ps_pv")
                for nkc in range(NKC):
                    mm(
                        ps_pv[:],
                        v_sb[:, nkc, hl * 65 : (hl + 1) * 65],
                        exps[i][:, nkc, :],
                        start=(nkc == 0),
                        stop=(nkc == NKC - 1),
                    )
                if i == 0:
                    nc.vector.tensor_copy(ctxT_sb[0:64, hp, qsl], ps_pv[0:DH, :])
                else:
                    ctmp = misc.tile([64, NQB], BF16, tag="ctmp", name="ctmp", bufs=2)
                    nc.vector.tensor_copy(ctmp[:], ps_pv[0:DH, :])
                    nc.sync.dma_start(ctxT_sb[64:128, hp, qsl], ctmp[:])
                # den row: PSUM partition 64 -> SBUF partition 64 (same lane),
                # then DMA shifts it into den_pair[i]
                dstage = misc.tile([65, NQB], F32, tag="dstage", name="dstage", bufs=2)
                nc.vector.tensor_copy(dstage[64:65, :], ps_pv[64:65, :])
                nc.sync.dma_start(den_pair[i : i + 1, :], dstage[64:65, :])
            # normalize this pair in place; overlaps the next pair's compute
            qsl2 = qsl
            recip_pair = den_pool.tile([2, NQB], F32, tag="recip", name="recip_pair")
            nc.vector.reciprocal(recip_pair[:], den_pair[:])
            rd = recip_drams[nqb][hp]
            nc.sync.dma_start(rd[:, :], recip_pair[:])
            rbc = misc.tile([P, NQB], F32, tag="rbc", name="rbc", bufs=2)
            nc.sync.dma_start(rbc[0:64, :], rd[0:1, :].to_broadcast((64, NQB)))
            nc.sync.dma_start(rbc[64:128, :], rd[1:2, :].to_broadcast((64, NQB)))
            nc.vector.tensor_tensor(
                ctxT_sb[:, hp, qsl2],
                ctxT_sb[:, hp, qsl2],
                rbc[:],
                mybir.AluOpType.mult,
            )

        def finish_block(nqb, psum_o):
            """out projection of one row block into its RS chunk(s)."""
            for lq in range(NQB // P):
                nqc = nqb * (NQB // P) + lq
                if nqb < NQBS - 1:
                    chunk, crow = chunks[nqb], lq * P
                elif lq < 2:
                    chunk, crow = chunks[3], lq * P
                else:
                    chunk, crow = chunks[4], (lq - 2) * P
                for cb in range(D // NQB):
                    ps = psum_o.tile([P, NQB], F32, tag="o", name="ps_o")
                    for fc in range(FO):
                        mm(
                            ps[:],
                            ctxT_sb[:, fc, nqc * P : (nqc + 1) * P],
                            wo_sb[:, fc, cb * NQB : (cb + 1) * NQB],
                            start=(fc == 0),
                            stop=(fc == FO - 1),
                        )
                    ob = outsb.tile([P, NQB], BF16, tag="ob", name="ob", bufs=2)
                    nc.vector.tensor_copy(ob[:], ps[:])
                    nc.sync.dma_start(
                        chunk[crow : crow + P, cb * NQB : (cb + 1) * NQB],
                        ob[:],
                    )
                if nqb == NQBS - 1 and lq == 1:
                    reduce_scatter(3)

        rs_row0 = [0, 256, 512, 768, 896]

        def reduce_scatter(q):
            nc.gpsimd.collective_compute(
                "ReduceScatter",
                mybir.AluOpType.add,
                replica_groups=[[0, 1], [2, 3], [4, 5], [6, 7]],
                ins=[chunks[q].opt()],
                outs=[rs_outs[q].opt()],
            )
            # gpsimd queue: a sync-queue DMA here would head-of-line block all
            # later sync DMAs behind the RS wait
            nc.gpsimd.dma_start(
                out_ext[rs_row0[q] : rs_row0[q] + chunk_rows[q] // 2, :],
                rs_outs[q][:],
            )

        # ---- Phase 1 (projections), interleaved with block-0 attention ----
        ph1_es = ExitStack()
        ph1 = ph1_es.enter_context(tc.tile_pool(name="ph1", bufs=1))
        psum1 = ph1_es.enter_context(
            tc.tile_pool(name="ph1_psum", bufs=2, space="PSUM")
        )
        # per-ko xT tiles: the first projection matmul only waits for the
        # first 512 KB slice instead of the whole 4 MB load
        wk_sb = ph1.tile([P, KO, GF], BF16, name="wk_sb")
        nc.sync.dma_start(wk_sb[:], wk.rearrange("(ko p) f -> p ko f", p=P))
        xT4 = xT.rearrange("(ko p) n -> ko p n", p=P)
        xT_k = []
        for ko in range(KO):
            t = ph1.tile([P, N], BF16, name=f"xT_k{ko}")
            nc.sync.dma_start(t[:], xT4[ko])
            xT_k.append(t)
        wq_sb = ph1.tile([P, KO, GF], BF16, name="wq_sb")
        nc.sync.dma_start(wq_sb[:], wq.rearrange("(ko p) f -> p ko f", p=P))
        wv_sb = ph1.tile([P, KO, GF], BF16, name="wv_sb")
        nc.sync.dma_start(wv_sb[:], wv.rearrange("(ko p) f -> p ko f", p=P))

        def proj_qk(fc):
            # bias rides a K=1 matmul chunk (lhsT = bias row, rhs = ones row);
            # the q-side 1/8 scale is folded into wq/bq8 on the host
            for w_sb, b_sb, dst in (
                (wk_sb, bk_sb, kT_f[fc]),
                (wq_sb, bq8_sb, qT_f[fc]),
            ):
                for nb in range(NQBS):
                    ps = psum1.tile([P, NQB], F32, tag="proj", name="ps_proj")
                    for ko in range(KO):
                        mm(
                            ps[:],
                            w_sb[:, ko, fc * P : (fc + 1) * P],
                            xT_k[ko][:, nb * NQB : (nb + 1) * NQB],
                            start=(ko == 0),
                            stop=False,
                        )
                    mm(
                        ps[:],
                        b_sb[:, fc * P : (fc + 1) * P],
                        ones_row[:, nb * NQB : (nb + 1) * NQB],
                        start=False,
                        stop=True,
                    )
                    nc.vector.tensor_copy(
                        dst[:, nb * NQB : (nb + 1) * NQB], ps[:]
                    )

        proj_qk(0)
        # v natural: [n, 512] = xT-chunk^T @ Wv, 65-wide per head (ones col)
        v4 = v_sb.rearrange("p nk (h e) -> p nk h e", e=DH + 1)
        nc.vector.memset(v4[:, :, :, DH], 1.0)
        for nk in range(NKC):
            ps = psum1.tile([P, GF], F32, tag="proj", name="ps_v")
            for ko in range(KO):
                mm(
                    ps[:],
                    xT_k[ko][:, nk * P : (nk + 1) * P],
                    wv_sb[:, ko, :],
                    start=(ko == 0),
                    stop=(ko == KO - 1),
                )
            nc.vector.tensor_copy(
                v4[:, nk, :, :DH],
                ps.rearrange("p (h e) -> p h e", e=DH),
            )

        for hp in range(HG // 2):
            attention_block(0, hp)
            if hp + 1 < FO:
                proj_qk(hp + 1)
        ph1_es.close()

        # out-projection PSUM pool opens after the projection pool closed
        psum_o = att_es.enter_context(
            tc.tile_pool(name="out_ps", bufs=2, space="PSUM")
        )
        finish_block(0, psum_o)
        reduce_scatter(0)

        for nqb in range(1, NQBS):
            for hp in range(HG // 2):
                attention_block(nqb, hp)
            finish_block(nqb, psum_o)
            reduce_scatter(nqb if nqb < NQBS - 1 else 4)
        att_es.close()


_NC_CACHE = None


def _get_nc():
    global _NC_CACHE
    if _NC_CACHE is None:
        _NC_CACHE = _build_nc()
    return _NC_CACHE


# --------------------------------------------------------------------------
# Timing support (test-only): build the sharded jit once, rerun on
# device-resident inputs, and subtract the axon dispatch floor measured on a
# trivial kernel.
# --------------------------------------------------------------------------


def _build_trivial_nc():
    nc = bacc.Bacc("TRN2", target_bir_lowering=False, debug=False,
                   num_devices=NCORES)
    tin = nc.dram_tensor("tin", [P, P], F32, kind="ExternalInput")
    tout = nc.dram_tensor("tout", [P, P], F32, kind="ExternalOutput")
    with tile.TileContext(nc) as tc:
        with tc.tile_pool(name="t", bufs=1) as pool:
            t = pool.tile([P, P], F32, name="t")
            nc.sync.dma_start(t[:], tin[:, :])
            nc.sync.dma_start(tout[:, :], t[:])
    nc.finalize()
    return nc


class _SpmdRunner:
    """Mirror of bass2jax.run_bass_via_pjrt's multi-core path with a cached
    jit so repeat executions don't recompile."""

    def __init__(self, nc):
        import jax
        from jax.sharding import Mesh, PartitionSpec
        try:
            from jax.experimental.shard_map import shard_map
        except ImportError:
            from jax.shard_map import shard_map
        from concourse import bass2jax as b2j

        b2j.install_neuronx_cc_hook()
        self.nc = nc
        partition_name = (
            nc.partition_id_tensor.name if nc.partition_id_tensor else None
        )
        in_names, out_names, out_avals, zero_outs = [], [], [], []
        for alloc in nc.m.functions[0].allocations:
            if not isinstance(alloc, mybir.MemoryLocationSet):
                continue
            name = alloc.memorylocations[0].name
            if alloc.kind == "ExternalInput":
                if name != partition_name:
                    in_names.append(name)
            elif alloc.kind == "ExternalOutput":
                shape = tuple(alloc.tensor_shape)
                dtype = mybir.dt.np(alloc.dtype)
                out_names.append(name)
                out_avals.append(jax.core.ShapedArray(shape, dtype))
                zero_outs.append(np.zeros(shape, dtype))
        self.n_params = len(in_names)
        n_outs = len(out_avals)
        in_names = in_names + out_names
        if partition_name is not None:
            in_names.append(partition_name)
        self.in_names = in_names
        self.out_names = out_names
        self.out_avals = out_avals
        self.zero_outs = zero_outs

        def _body(*args):
            operands = list(args)
            if partition_name is not None:
                operands.append(b2j.partition_id_tensor())
            outs = b2j._bass_exec_p.bind(
                *operands,
                out_avals=tuple(out_avals),
                in_names=tuple(in_names),
                out_names=tuple(out_names),
                lowering_input_output_aliases=(),
                sim_require_finite=True,
                sim_require_nnan=True,
                nc=nc,
            )
            return tuple(outs)

        devices = jax.devices()[:NCORES]
        self.mesh = Mesh(np.asarray(devices), ("core",))
        in_specs = (PartitionSpec("core"),) * (self.n_params + n_outs)
        out_specs = (PartitionSpec("core"),) * n_outs
        self.fn = jax.jit(
            shard_map(_body, mesh=self.mesh, in_specs=in_specs,
                      out_specs=out_specs, check_rep=False),
            keep_unused=True,
        )
        self._jax = jax

    def make_fn_k(self, K):
        """jit that executes the NEFF K times; all outputs kept live so the
        calls can't be DCE'd. Used to amortize the ~78 ms axon dispatch floor
        out of timing: exec_ns ~= (t(K) - t(1)) / (K - 1)."""
        import jax
        from jax.sharding import PartitionSpec
        try:
            from jax.experimental.shard_map import shard_map
        except ImportError:
            from jax.shard_map import shard_map
        from concourse import bass2jax as b2j

        nc = self.nc
        partition_name = nc.partition_id_tensor.name if nc.partition_id_tensor else None
        in_names, out_names, out_avals = self.in_names, self.out_names, self.out_avals

        def _body_k(*args):
            all_outs = []
            for _ in range(K):
                operands = list(args)
                if partition_name is not None:
                    operands.append(b2j.partition_id_tensor())
                outs = b2j._bass_exec_p.bind(
                    *operands,
                    out_avals=tuple(out_avals),
                    in_names=tuple(in_names),
                    out_names=tuple(out_names),
                    lowering_input_output_aliases=(),
                    sim_require_finite=True,
                    sim_require_nnan=True,
                    nc=nc,
                )
                all_outs.extend(outs)
            return tuple(all_outs)

        n_outs = len(out_avals)
        in_specs = (PartitionSpec("core"),) * (self.n_params + n_outs)
        out_specs = (PartitionSpec("core"),) * (n_outs * K)
        return jax.jit(
            shard_map(_body_k, mesh=self.mesh, in_specs=in_specs,
                      out_specs=out_specs, check_rep=False),
            keep_unused=True,
        )

    def time_k(self, in_maps, K=8, reps=12):
        import time as _time

        dev_in, dev_zero = self.prepare(in_maps)
        fn_k = self.make_fn_k(K)
        fn_1 = self.make_fn_k(1)
        for fn in (fn_1, fn_k):
            self._jax.block_until_ready(fn(*dev_in, *dev_zero))  # compile+warm
        t1s, tks = [], []
        for _ in range(reps):
            t0 = _time.perf_counter()
            self._jax.block_until_ready(fn_1(*dev_in, *dev_zero))
            t1s.append(_time.perf_counter() - t0)
            t0 = _time.perf_counter()
            self._jax.block_until_ready(fn_k(*dev_in, *dev_zero))
            tks.append(_time.perf_counter() - t0)
        t1, tk = min(t1s), min(tks)
        return (tk - t1) / (K - 1), t1, tk

    def _shard(self, arrs):
        import jax
        from jax.sharding import NamedSharding, PartitionSpec

        sh = NamedSharding(self.mesh, PartitionSpec("core"))
        return [jax.device_put(a, sh) for a in arrs]

    def prepare(self, in_maps):
        concat_in = [
            np.concatenate([np.asarray(m[name]) for m in in_maps], axis=0)
            for name in self.in_names[: self.n_params]
        ]
        concat_zeros = [
            np.zeros((NCORES * z.shape[0], *z.shape[1:]), z.dtype)
            for z in self.zero_outs
        ]
        return self._shard(concat_in), self._shard(concat_zeros)

    def run(self, dev_in, dev_zero):
        outs = self.fn(*dev_in, *dev_zero)
        self._jax.block_until_ready(outs)
        return outs

    def time(self, in_maps, reps=10):
        import time as _time

        dev_in, dev_zero = self.prepare(in_maps)
        self.run(dev_in, dev_zero)  # warm/compile
        ts = []
        for _ in range(reps):
            t0 = _time.perf_counter()
            self.run(dev_in, dev_zero)
            ts.append(_time.perf_counter() - t0)
        return min(ts), ts

    def results(self, in_maps):
        dev_in, dev_zero = self.prepare(in_maps)
        outs = self.run(dev_in, dev_zero)
        res = []
        for c in range(NCORES):
            res.append(
                {
                    name: np.asarray(outs[i]).reshape(
                        NCORES, *self.out_avals[i].shape
                    )[c]
                    for i, name in enumerate(self.out_names)
                }
            )
        return res


_RUNNER = None
_TRIVIAL_RUNNER = None


def get_runner():
    global _RUNNER
    if _RUNNER is None:
        _RUNNER = _SpmdRunner(_get_nc())
    return _RUNNER


def get_trivial_runner():
    global _TRIVIAL_RUNNER
    if _TRIVIAL_RUNNER is None:
        _TRIVIAL_RUNNER = _SpmdRunner(_build_trivial_nc())
    return _TRIVIAL_RUNNER


def make_in_maps(x, Wq, bq, Wk, bk, Wv, bv, Wo, bo):
    x = np.asarray(x, np.float32)
    in_maps = []
    for core in range(NCORES):
        b, g = core // 2, core % 2
        gsl = slice(g * GF, (g + 1) * GF)
        in_maps.append(
            {
                "xT": np.ascontiguousarray(x[b].T).astype(BF16_NP),
                "wq": np.ascontiguousarray(np.asarray(Wq)[:, gsl] / 8.0).astype(
                    BF16_NP
                ),
                "wk": np.ascontiguousarray(np.asarray(Wk)[:, gsl]).astype(BF16_NP),
                "wv": np.ascontiguousarray(np.asarray(Wv)[:, gsl]).astype(BF16_NP),
                "wo": np.ascontiguousarray(np.asarray(Wo)[gsl, :]).astype(BF16_NP),
                "bq8": np.ascontiguousarray(np.asarray(bq)[gsl] / 8.0).astype(
                    BF16_NP
                ),
                "bk": np.ascontiguousarray(np.asarray(bk)[gsl]).astype(BF16_NP),
            }
        )
    return in_maps


def kernel(x, Wq, bq, Wk, bk, Wv, bv, Wo, bo):
    Wo = np.asarray(Wo, np.float32)
    bv = np.asarray(bv, np.float32)
    bo = np.asarray(bo, np.float32)
    in_maps = make_in_maps(x, Wq, bq, Wk, bk, Wv, bv, Wo, bo)
    results = get_runner().results(in_maps)

    post = (bv @ Wo + bo).astype(np.float32)  # softmax rows sum to 1 -> bv folds
    # Per-chunk RS: rank g keeps the g-th half of each chunk's rows.
    # Segments: (global row start for rank 0, rows per rank).
    segments = [(0, 256), (512, 256), (1024, 256), (1536, 128), (1792, 128)]
    out = np.empty((B, N, D), np.float32)
    for b in range(B):
        for g in range(2):
            r = np.asarray(results[2 * b + g]["out"], np.float32)
            ofs = 0
            for g0, nrows in segments:
                out[b, g0 + g * nrows : g0 + (g + 1) * nrows] = r[ofs : ofs + nrows]
                ofs += nrows
        out[b] += post
    return out
